# revision 50
# baseline (speedup 1.0000x reference)
"""Trainium2 Bass kernel for nn_Aspp_Attention: ASPP-KV attention over 2D features.

Sharding: pure data-parallel — batch b=8 over 8 NeuronCores, one image per core.
Device dataflow per core (x: (128 c, 16384 hw) f32):
  pools run on raw x (pos pool-sums folded host-side); depthwise3x3+pointwise
  fused into 36 accumulating PE matmuls (per-level scale dropped: LN-invariant);
  LN -> gelu -> z2 (c,85) bf16; A = [0.25*Wq_m^T Wk_m] z2 (c,768 bf16, padded).
  xp = x + pos adds (DVE) stream into the first loop iterations.
  Loop per token group (512), chunk-pairs: scores^T = A_cc^T @ xp (PE bf16),
  exp on ACT (PSUM->SBUF bf16), H += blockdiag(v)^T exp, r128 += ones2^T exp (PE),
  rec = 1/r128, hn = H*rec (DVE); tail (Wproj MM + bias store + DMA) pipelined
  one group behind so the PE never waits on the DVE chain.
"""
import os
from contextlib import ExitStack

import numpy as np

B, C, Hh, Ww = 8, 128, 128, 128
HW = Hh * Ww
M, HD, KV = 8, 16, 85
KVH = M * KV  # 680
CH_B = [0, 128, 256, 384, 512, 640, 680]
NCH = 6
G = 512            # token group
NG = HW // G       # 32

_CACHE = {}


def _pos_full():
    ch = 64
    inv = 1.0 / (10000.0 ** (np.arange(0, ch, 2, dtype=np.float32) / ch))
    px = np.arange(Hh, dtype=np.float32)[:, None] * inv
    ex = np.concatenate([np.sin(px), np.cos(px)], -1).astype(np.float32)  # (128,64)
    pos = np.zeros((C, Hh, Ww), np.float32)
    pos[:64] = ex.T[:, :, None]
    pos[64:] = ex.T[:, None, :]
    return pos.reshape(C, HW)


def _build():
    import concourse.bass as bass
    import concourse.bacc as bacc
    import concourse.tile as tile
    from concourse import mybir

    nc = bacc.Bacc()
    f32 = mybir.dt.float32
    bf16 = mybir.dt.bfloat16
    f8 = mybir.dt.float8e4
    DR = mybir.MatmulPerfMode.DoubleRow
    AF = mybir.ActivationFunctionType
    AX = mybir.AxisListType

    x_d = nc.dram_tensor("x", [C, HW], f32, kind="ExternalInput")
    pos_d = nc.dram_tensor("pos", [C, HW], bf16, kind="ExternalInput")
    ct_d = nc.dram_tensor("ct", [M * C, C], bf16, kind="ExternalInput")  # lhsT for A
    wvt_d = nc.dram_tensor("wvt", [C, C], bf16, kind="ExternalInput")
    pwdw_d = nc.dram_tensor("pwdw", [9 * C, C], bf16, kind="ExternalInput")
    wpt_d = nc.dram_tensor("wpt", [C, C], bf16, kind="ExternalInput")
    pp_d = nc.dram_tensor("pospool", [C, KV], f32, kind="ExternalInput")
    ones_d = nc.dram_tensor("onesb", [C, NCH * C], f8, kind="ExternalInput")
    lnw_d = nc.dram_tensor("lnw", [C, 1], f32, kind="ExternalInput")
    lnb_d = nc.dram_tensor("lnb", [C, 1], f32, kind="ExternalInput")
    bpj_d = nc.dram_tensor("bpj", [C, 1], f32, kind="ExternalInput")
    idn_d = nc.dram_tensor("idn", [C, C], f32, kind="ExternalInput")
    out_d = nc.dram_tensor("out", [C, HW], f32, kind="ExternalOutput")

    with ExitStack() as ctx:
        tc = ctx.enter_context(tile.TileContext(nc))
        singles = ctx.enter_context(tc.tile_pool(name="singles", bufs=1))
        xpool = ctx.enter_context(tc.tile_pool(name="xp", bufs=1))
        exp_pool = ctx.enter_context(tc.tile_pool(name="exp", bufs=3))
        outp = ctx.enter_context(tc.tile_pool(name="outs", bufs=5))
        rr_pool = ctx.enter_context(tc.tile_pool(name="rr", bufs=2))
        ps_sc = ctx.enter_context(tc.tile_pool(name="psS", bufs=2, space="PSUM"))
        ps_h = ctx.enter_context(tc.tile_pool(name="psH", bufs=1, space="PSUM"))
        ps_r = ctx.enter_context(tc.tile_pool(name="psR", bufs=2, space="PSUM"))
        ps_ro = ctx.enter_context(tc.tile_pool(name="psRO", bufs=1, space="PSUM"))

        dmae = [nc.sync, nc.scalar, nc.gpsimd]

        # ---- z-chain consts first (tiny, gate the kv path) on the deep
        # gpsimd queue so they never sit behind the bulk x stream
        pwdw_sb = singles.tile([C, 9 * C], bf16)    # pwdw_sb[:, t*C:] = lhsT_t
        for t in range(9):
            nc.gpsimd.dma_start(out=pwdw_sb[:, t * C:(t + 1) * C],
                                in_=pwdw_d[t * C:(t + 1) * C, :])
        pp_sb = singles.tile([C, KV], f32)
        nc.gpsimd.dma_start(out=pp_sb, in_=pp_d[:, :])
        idn_sb = singles.tile([C, C], f32)
        nc.gpsimd.dma_start(out=idn_sb, in_=idn_d[:, :])
        wvt_sb = singles.tile([C, C], bf16)
        nc.gpsimd.dma_start(out=wvt_sb, in_=wvt_d[:, :])
        lnw_sb = singles.tile([C, 1], f32)
        nc.gpsimd.dma_start(out=lnw_sb, in_=lnw_d[:, :])
        lnb_sb = singles.tile([C, 1], f32)
        nc.gpsimd.dma_start(out=lnb_sb, in_=lnb_d[:, :])

        # ---- stream ALL of x next (pools gate the whole kv path); pos is
        # only needed by the loop's xp adds, so it streams after the consts.
        NXC = 16
        xc = HW // NXC
        s1 = singles.tile([C, Hh, 8], f32)   # x summed over w-blocks of 16
        xst = []
        pst = []
        for i in range(NXC):
            xt = singles.tile([C, xc], f32, tag=f"xin{i}", name=f"xin{i}")
            dmae[i % 3].dma_start(out=xt, in_=x_d[:, i * xc:(i + 1) * xc])
            xst.append(xt)
            nc.vector.reduce_sum(
                s1[:, i * 8:(i + 1) * 8, :],
                xt.rearrange("c (h wg wi) -> c h wg wi", wg=8, wi=16), axis=AX.X)

        # ---- loop-side consts (needed only once the loop starts)
        ct_sb = singles.tile([C, M * C], bf16)      # ct_sb[:, m*C:(m+1)*C] = CT_m
        for m in range(M):
            nc.gpsimd.dma_start(out=ct_sb[:, m * C:(m + 1) * C],
                                in_=ct_d[m * C:(m + 1) * C, :])
        wpt_sb = singles.tile([C, C], bf16)
        nc.gpsimd.dma_start(out=wpt_sb, in_=wpt_d[:, :])
        ones_sb = singles.tile([C, NCH * C], f8)
        nc.gpsimd.dma_start(out=ones_sb, in_=ones_d[:, :])
        bpj_sb = singles.tile([C, 1], f32)
        nc.gpsimd.dma_start(out=bpj_sb, in_=bpj_d[:, :])

        # pos chunk tiles: first 3 dedicated, rest rotate through the out
        # pool so each pos DMA waits (WAR) for an earlier out-store to drain
        # -- keeps the pos stream off the HBM wire during the x prelude
        for i in range(3):
            pt = singles.tile([C, xc], bf16, tag=f"pin{i}", name=f"pin{i}")
            pst.append(pt)

        def pos_dma(i):
            nc.gpsimd.dma_start(out=pst[i], in_=pos_d[:, i * xc:(i + 1) * xc])

        # ---- remaining pool levels (sums) + host-folded pos pool sums
        p8 = singles.tile([C, 8, 8], f32)
        nc.vector.reduce_sum(
            p8, s1.rearrange("c (hg hi) wg -> c hg wg hi", hi=16), axis=AX.X)
        p4 = singles.tile([C, 4, 4], f32)
        t44 = singles.tile([C, 8, 4], f32)
        nc.vector.reduce_sum(t44, p8.rearrange("c h (wg wi) -> c h wg wi", wi=2), axis=AX.X)
        nc.vector.reduce_sum(p4, t44.rearrange("c (hg hi) w -> c hg w hi", hi=2), axis=AX.X)
        p2 = singles.tile([C, 2, 2], f32)
        t22 = singles.tile([C, 4, 2], f32)
        nc.vector.reduce_sum(t22, p4.rearrange("c h (wg wi) -> c h wg wi", wi=2), axis=AX.X)
        nc.vector.reduce_sum(p2, t22.rearrange("c (hg hi) w -> c hg w hi", hi=2), axis=AX.X)
        p1 = singles.tile([C, 1, 1], f32)
        t11 = singles.tile([C, 2, 1], f32)
        nc.vector.reduce_sum(t11, p2.rearrange("c h (wg wi) -> c h wg wi", wi=2), axis=AX.X)
        nc.vector.reduce_sum(p1, t11.rearrange("c (hg hi) w -> c hg w hi", hi=2), axis=AX.X)
        offs = {8: 0, 4: 64, 2: 80, 1: 84}
        for s, ps in ((8, p8), (4, p4), (2, p2), (1, p1)):
            o = offs[s]
            psl = pp_sb[:, o:o + s * s].rearrange("c (h w) -> c h w", h=s)
            nc.vector.tensor_add(ps, ps, psl)

        # ---- fused depthwise+pointwise: z1 = sum_t PWdiag(tap_t) @ pad_shift_t
        # (per-level 1/blk scale dropped -- LN normalizes it out)
        z1_ps = ps_ro.tile([C, KV], f32, tag="ro")
        for lvl, (s, ps) in enumerate(((8, p8), (4, p4), (2, p2), (1, p1))):
            pad = singles.tile([C, (s + 2) * (s + 2)], bf16, tag=f"pad{s}")
            nc.vector.memset(pad, 0.0)
            pad3 = pad.rearrange("c (h w) -> c h w", h=s + 2)
            nc.vector.tensor_copy(pad3[:, 1:s + 1, 1:s + 1], ps)
            o = offs[s]
            dst = z1_ps[:, o:o + s * s].rearrange("c (h w) -> c h w", h=s)
            for di in range(3):
                for dj in range(3):
                    t = 3 * di + dj
                    nc.tensor.matmul(dst, lhsT=pwdw_sb[:, t * C:(t + 1) * C],
                                     rhs=pad3[:, di:di + s, dj:dj + s],
                                     start=(t == 0), stop=(t == 8))
        z1_sb = singles.tile([C, KV], f32)
        nc.scalar.copy(z1_sb, z1_ps)

        # ---- LN over c: transpose -> stats -> zn -> transpose back -> gelu
        zt_ps = ps_ro.tile([KV, C], f32, tag="ro")
        nc.tensor.transpose(zt_ps, z1_sb, idn_sb)
        zt_sb = singles.tile([KV, C], f32)
        nc.scalar.copy(zt_sb, zt_ps)
        nmu = singles.tile([KV, 1], f32)
        nc.vector.reduce_sum(nmu, zt_sb, axis=AX.X, negate=True)
        nc.vector.tensor_scalar_mul(nmu, nmu, 1.0 / C)
        zc = singles.tile([KV, C], f32)
        nc.vector.tensor_scalar_add(zc, zt_sb, nmu)
        sq = singles.tile([KV, C], f32)
        nc.vector.tensor_mul(sq, zc, zc)
        var = singles.tile([KV, 1], f32)
        nc.vector.reduce_sum(var, sq, axis=AX.X)
        std = singles.tile([KV, 1], f32)
        eps_sb = singles.tile([KV, 1], f32)
        nc.vector.memset(eps_sb, 1e-5)
        nc.scalar.activation(std, var, AF.Sqrt, bias=eps_sb, scale=1.0 / C)
        rstd = singles.tile([KV, 1], f32)
        nc.vector.reciprocal(rstd, std)
        zn = singles.tile([KV, C], f32)
        nc.vector.tensor_scalar_mul(zn, zc, rstd)
        znt_ps = ps_ro.tile([C, KV], f32, tag="ro")
        nc.tensor.transpose(znt_ps, zn, idn_sb[:KV, :KV])
        z2 = singles.tile([C, KV], bf16)
        nc.scalar.activation(z2, znt_ps, AF.Gelu, bias=lnb_sb, scale=lnw_sb)

        # ---- vkv (85, 128) bf16 + b2 blockdiag first (H MMs depend on it),
        # then A (c, 768 zero-padded)
        vt_ps = ps_ro.tile([C, KV], f32, tag="ro")
        nc.tensor.matmul(vt_ps, lhsT=wvt_sb, rhs=z2, start=True, stop=True)
        vt_sb = singles.tile([C, KV], f32)
        nc.scalar.copy(vt_sb, vt_ps)
        vkv_ps = ps_ro.tile([KV, C], f32, tag="ro")
        nc.tensor.transpose(vkv_ps, vt_sb, idn_sb)
        vkv_sb = singles.tile([KV, C], f8)
        nc.scalar.copy(vkv_sb, vkv_ps)

        b2_sb = singles.tile([C, NCH * C], f8)
        nc.vector.memset(b2_sb, 0.0)
        nq = 0
        for m in range(M):
            g0, g1 = KV * m, KV * (m + 1)
            for cchunk in range(NCH):
                c0, c1 = CH_B[cchunk], CH_B[cchunk + 1]
                lo, hi = max(g0, c0), min(g1, c1)
                if lo >= hi:
                    continue
                nc.gpsimd.dma_start(
                    out=b2_sb[lo - c0:hi - c0,
                              cchunk * C + HD * m: cchunk * C + HD * m + HD],
                    in_=vkv_sb[lo - g0:hi - g0, HD * m:HD * m + HD])
                nq += 1

        a_sb = singles.tile([C, NCH * C], bf16)
        nc.vector.memset(a_sb[:, KVH:], 0.0)
        for half in range(2):
            a_ps = ps_ro.tile([C, 4 * KV], f32, tag="ro")
            for mi in range(4):
                m = half * 4 + mi
                nc.tensor.matmul(a_ps[:, mi * KV:(mi + 1) * KV],
                                 lhsT=ct_sb[:, m * C:(m + 1) * C], rhs=z2,
                                 start=True, stop=True)
            nc.scalar.copy(a_sb[:, half * 4 * KV:(half + 1) * 4 * KV], a_ps)

        # ---- main attention loop, flat pipeline over 96 chunk-pairs:
        # scores/exp of pair P are emitted BEFORE H/r of pair P-1, so the PE
        # FIFO always has ready score work while ACT runs exp. Wproj/store
        # tail runs one group behind; xp adds stream into iterations 0..15.
        xp_sb = xpool.tile([C, HW], bf16)
        for i in range(3):
            pos_dma(i)
        GH = 1024
        NP = 3 * NG
        gstate = {}
        pend = [None]

        def emit_tail2():
            hn_, tt = pend[0]
            o_ps = ps_ro.tile([C, G], f32, tag="ro")
            nc.tensor.matmul(o_ps, lhsT=wpt_sb, rhs=hn_, start=True, stop=True)
            o_sb = outp.tile([C, G], f32)
            nc.vector.tensor_scalar_add(o_sb, o_ps, bpj_sb)
            nc.gpsimd.dma_start(out=out_d[:, tt:tt + G], in_=o_sb)
            pend[0] = None

        def emit_scores_exp(P):
            g, pp = divmod(P, 3)
            if pp == 0:
                if g + 3 < 16:
                    pst.append(outp.tile([C, xc], bf16, tag="o",
                                         name=f"pin{g + 3}"))
                    pos_dma(g + 3)
                # one 512-token add per group keeps DVE load flat at
                # ~0.7us/group instead of 1.2us spikes in the first half
                nc.vector.tensor_add(xp_sb[:, g * G:(g + 1) * G],
                                     xst[g // 2][:, (g % 2) * G:(g % 2 + 1) * G],
                                     pst[g // 2][:, (g % 2) * G:(g % 2 + 1) * G])
                gstate[g] = (ps_h.tile([C, G], f32, tag="h", name=f"h{g}"),
                             ps_r.tile([C, G], f32, tag="r", name=f"r{g}"),
                             exp_pool.tile([C, NCH * G], f8, tag="ex",
                                           name=f"ex{g}"))
            h_ps, r_ps, ex_sb = gstate[g]
            xg = xp_sb[:, g * G:(g + 1) * G]
            s_ps = ps_sc.tile([C, 2 * G], f32, tag="s")
            for j in range(2):
                cc = 2 * pp + j
                nc.tensor.matmul(s_ps[:, j * G:(j + 1) * G],
                                 lhsT=a_sb[:, cc * C:(cc + 1) * C], rhs=xg,
                                 start=True, stop=True)
            if pp == 1 and pend[0] is not None:
                emit_tail2()
            nc.scalar.activation(ex_sb[:, 2 * pp * G:(2 * pp + 2) * G], s_ps,
                                 AF.Exp)

        def emit_hr(P):
            g, pp = divmod(P, 3)
            h_ps, r_ps, ex_sb = gstate[g]
            erhs = ex_sb[:, 2 * pp * G:(2 * pp + 2) * G].rearrange(
                "k (two t) -> k two t", two=2)
            bv = b2_sb[:, 2 * pp * C:(2 * pp + 2) * C].rearrange(
                "k (two m) -> k two m", two=2)
            ov = ones_sb[:, 2 * pp * C:(2 * pp + 2) * C].rearrange(
                "k (two m) -> k two m", two=2)
            nc.tensor.matmul(h_ps, lhsT=bv, rhs=erhs, perf_mode=DR,
                             start=(pp == 0), stop=(pp == 2))
            nc.tensor.matmul(r_ps, lhsT=ov, rhs=erhs, perf_mode=DR,
                             start=(pp == 0), stop=(pp == 2))
            if pp == 2:
                rec = rr_pool.tile([C, G], f32, tag="rec")
                nc.vector.reciprocal_approx_fast(rec, r_ps)
                hn = rr_pool.tile([C, G], bf16, tag="hn")
                nc.vector.tensor_mul(hn, h_ps, rec)
                pend[0] = (hn, g * G)
                del gstate[g]

        for P in range(NP + 1):
            if P < NP:
                emit_scores_exp(P)
            if P >= 1:
                emit_hr(P - 1)
        emit_tail2()

    nc.finalize()
    return nc


def _consts(Wq, Wkv, Wproj, bproj, dw_w, pw_w, ln_w, ln_b):
    import ml_dtypes

    bf16 = ml_dtypes.bfloat16
    scale = HD ** -0.5
    Wk, Wv = Wkv[:128], Wkv[128:]
    ct = np.zeros((M * C, C), np.float32)
    for m in range(M):
        ct[m * C:(m + 1) * C] = scale * Wk[16 * m:16 * m + 16].T @ Wq[16 * m:16 * m + 16]
    # fused pointwise*diag(dw tap t), transposed for lhsT
    pw = pw_w[:, :, 0, 0]                      # (C out, C in)
    taps = dw_w[:, 0].reshape(C, 9)            # (C, 9)
    pwdw = np.zeros((9 * C, C), np.float32)
    for t in range(9):
        pwdw[t * C:(t + 1) * C] = pw.T * taps[:, t:t + 1]
    # pos pool sums per level, level-major like dwcat
    pos = _pos_full().reshape(C, Hh, Ww)
    pospool = np.zeros((C, KV), np.float32)
    offs = {8: 0, 4: 64, 2: 80, 1: 84}
    for s in (8, 4, 2, 1):
        blk = Hh // s
        psum = pos.reshape(C, s, blk, s, blk).sum((2, 4))
        pospool[:, offs[s]:offs[s] + s * s] = psum.reshape(C, s * s)
    # expanded ones: chunk cc row r hits all 16 channels of its head
    onesb = np.zeros((C, NCH * C), np.float32)
    for cc in range(NCH):
        c0, c1 = CH_B[cc], CH_B[cc + 1]
        for r in range(c1 - c0):
            m = (c0 + r) // KV
            onesb[r, cc * C + HD * m: cc * C + HD * (m + 1)] = 1.0
    return {
        "ct": ct.astype(bf16),
        "wvt": np.ascontiguousarray(Wv.T).astype(bf16),
        "pwdw": pwdw.astype(bf16),
        "wpt": np.ascontiguousarray(Wproj.T).astype(bf16),
        "pospool": pospool,
        "onesb": onesb.astype(ml_dtypes.float8_e4m3),
        "lnw": ln_w.reshape(C, 1).astype(np.float32),
        "lnb": ln_b.reshape(C, 1).astype(np.float32),
        "bpj": bproj.reshape(C, 1).astype(np.float32),
        "idn": np.eye(C, dtype=np.float32),
        "pos": _pos_full().astype(bf16),
    }


def kernel(x, Wq, Wkv, Wproj, bproj, dw_w, pw_w, ln_w, ln_b):
    from concourse.bass_utils import run_bass_kernel_spmd

    if "nc" not in _CACHE:
        _CACHE["nc"] = _build()
    nc = _CACHE["nc"]

    cst = _consts(np.asarray(Wq, np.float32), np.asarray(Wkv, np.float32),
                  np.asarray(Wproj, np.float32), np.asarray(bproj, np.float32),
                  np.asarray(dw_w, np.float32), np.asarray(pw_w, np.float32),
                  np.asarray(ln_w, np.float32), np.asarray(ln_b, np.float32))
    x = np.asarray(x, np.float32)
    in_maps = []
    for b in range(B):
        im = {"x": np.ascontiguousarray(x[b].reshape(C, HW))}
        im.update(cst)
        in_maps.append(im)

    trace = bool(int(os.environ.get("KPROF", "0")))
    res = run_bass_kernel_spmd(nc, in_maps, core_ids=list(range(B)), trace=trace)
    if trace and res.exec_time_ns is not None:
        print(f"HW exec time: {res.exec_time_ns} ns")
    out = np.stack([res.results[b]["out"].reshape(C, Hh, Ww) for b in range(B)])
    return out


# revision 52
# speedup vs baseline: 1.1504x; 1.1504x over previous
"""Trainium2 Bass kernel for nn_Aspp_Attention: ASPP-KV attention over 2D features.

Sharding: pure data-parallel — batch b=8 over 8 NeuronCores, one image per core.
Device dataflow per core (x: (128 c, 16384 hw) f32):
  pools run on raw x (pos pool-sums folded host-side); depthwise3x3+pointwise
  fused into 36 accumulating PE matmuls (per-level scale dropped: LN-invariant);
  LN -> gelu -> z2 (c,85) bf16; A = [0.25*Wq_m^T Wk_m] z2 (c,768 bf16, padded).
  xp = x + pos adds (DVE) stream into the first loop iterations.
  Loop per token group (512), chunk-pairs: scores^T = A_cc^T @ xp (PE bf16),
  exp on ACT (PSUM->SBUF bf16), H += blockdiag(v)^T exp, r128 += ones2^T exp (PE),
  rec = 1/r128, hn = H*rec (DVE); tail (Wproj MM + bias store + DMA) pipelined
  one group behind so the PE never waits on the DVE chain.
"""
import os
from contextlib import ExitStack

import numpy as np

B, C, Hh, Ww = 8, 128, 128, 128
HW = Hh * Ww
M, HD, KV = 8, 16, 85
KVH = M * KV  # 680
CH_B = [0, 128, 256, 384, 512, 640, 680]
NCH = 6
G = 512            # token group
NG = HW // G       # 32

_CACHE = {}


def _pos_full():
    ch = 64
    inv = 1.0 / (10000.0 ** (np.arange(0, ch, 2, dtype=np.float32) / ch))
    px = np.arange(Hh, dtype=np.float32)[:, None] * inv
    ex = np.concatenate([np.sin(px), np.cos(px)], -1).astype(np.float32)  # (128,64)
    pos = np.zeros((C, Hh, Ww), np.float32)
    pos[:64] = ex.T[:, :, None]
    pos[64:] = ex.T[:, None, :]
    return pos.reshape(C, HW)


def _build():
    import concourse.bass as bass
    import concourse.bacc as bacc
    import concourse.tile as tile
    from concourse import mybir

    nc = bacc.Bacc()
    f32 = mybir.dt.float32
    bf16 = mybir.dt.bfloat16
    f8 = mybir.dt.float8e4
    DR = mybir.MatmulPerfMode.DoubleRow
    AF = mybir.ActivationFunctionType
    AX = mybir.AxisListType

    x_d = nc.dram_tensor("x", [C, HW], f32, kind="ExternalInput")
    pos_d = nc.dram_tensor("pos", [C, HW], bf16, kind="ExternalInput")
    ct_d = nc.dram_tensor("ct", [M * C, C], bf16, kind="ExternalInput")  # lhsT for A
    wvt_d = nc.dram_tensor("wvt", [C, C], bf16, kind="ExternalInput")
    pwdw_d = nc.dram_tensor("pwdw", [9 * C, C], bf16, kind="ExternalInput")
    wpt_d = nc.dram_tensor("wpt", [C, C], bf16, kind="ExternalInput")
    pp_d = nc.dram_tensor("pospool", [C, KV], f32, kind="ExternalInput")
    ones_d = nc.dram_tensor("onesb", [C, NCH * C], f8, kind="ExternalInput")
    lnw_d = nc.dram_tensor("lnw", [C, 1], f32, kind="ExternalInput")
    lnb_d = nc.dram_tensor("lnb", [C, 1], f32, kind="ExternalInput")
    bpj_d = nc.dram_tensor("bpj", [C, 1], f32, kind="ExternalInput")
    idn_d = nc.dram_tensor("idn", [C, C], f32, kind="ExternalInput")
    out_d = nc.dram_tensor("out", [C, HW], f32, kind="ExternalOutput")

    with ExitStack() as ctx:
        tc = ctx.enter_context(tile.TileContext(nc))
        singles = ctx.enter_context(tc.tile_pool(name="singles", bufs=1))
        xpool = ctx.enter_context(tc.tile_pool(name="xp", bufs=1))
        exp_pool = ctx.enter_context(tc.tile_pool(name="exp", bufs=2))
        outp = ctx.enter_context(tc.tile_pool(name="outs", bufs=3))
        rr_pool = ctx.enter_context(tc.tile_pool(name="rr", bufs=3))
        ps_sc = ctx.enter_context(tc.tile_pool(name="psS", bufs=2, space="PSUM"))
        ps_h = ctx.enter_context(tc.tile_pool(name="psH", bufs=1, space="PSUM"))
        ps_r = ctx.enter_context(tc.tile_pool(name="psR", bufs=2, space="PSUM"))
        ps_ro = ctx.enter_context(tc.tile_pool(name="psRO", bufs=1, space="PSUM"))

        dmae = [nc.sync, nc.scalar, nc.gpsimd]

        # ---- z-chain consts first (tiny, gate the kv path) on the deep
        # gpsimd queue so they never sit behind the bulk x stream
        pwdw_sb = singles.tile([C, 9 * C], bf16)    # pwdw_sb[:, t*C:] = lhsT_t
        for t in range(9):
            nc.gpsimd.dma_start(out=pwdw_sb[:, t * C:(t + 1) * C],
                                in_=pwdw_d[t * C:(t + 1) * C, :])
        pp_sb = singles.tile([C, KV], f32)
        nc.gpsimd.dma_start(out=pp_sb, in_=pp_d[:, :])
        idn_sb = singles.tile([C, C], f32)
        nc.gpsimd.dma_start(out=idn_sb, in_=idn_d[:, :])
        wvt_sb = singles.tile([C, C], bf16)
        nc.gpsimd.dma_start(out=wvt_sb, in_=wvt_d[:, :])
        lnw_sb = singles.tile([C, 1], f32)
        nc.gpsimd.dma_start(out=lnw_sb, in_=lnw_d[:, :])
        lnb_sb = singles.tile([C, 1], f32)
        nc.gpsimd.dma_start(out=lnb_sb, in_=lnb_d[:, :])

        # ---- stream ALL of x next (pools gate the whole kv path); pos is
        # only needed by the loop's xp adds, so it streams after the consts.
        NXC = 16
        xc = HW // NXC
        s1 = singles.tile([C, Hh, 8], f32)   # x summed over w-blocks of 16
        xst = []
        pst = []
        for i in range(NXC):
            xt = singles.tile([C, xc], f32, tag=f"xin{i}", name=f"xin{i}")
            dmae[i % 3].dma_start(out=xt, in_=x_d[:, i * xc:(i + 1) * xc])
            xst.append(xt)
            nc.vector.reduce_sum(
                s1[:, i * 8:(i + 1) * 8, :],
                xt.rearrange("c (h wg wi) -> c h wg wi", wg=8, wi=16), axis=AX.X)

        # ---- loop-side consts (needed only once the loop starts)
        ct_sb = singles.tile([C, M * C], bf16)      # ct_sb[:, m*C:(m+1)*C] = CT_m
        for m in range(M):
            nc.gpsimd.dma_start(out=ct_sb[:, m * C:(m + 1) * C],
                                in_=ct_d[m * C:(m + 1) * C, :])
        wpt_sb = singles.tile([C, C], bf16)
        nc.gpsimd.dma_start(out=wpt_sb, in_=wpt_d[:, :])
        ones_sb = singles.tile([C, NCH * C], f8)
        nc.gpsimd.dma_start(out=ones_sb, in_=ones_d[:, :])
        bpj_sb = singles.tile([C, 1], f32)
        nc.gpsimd.dma_start(out=bpj_sb, in_=bpj_d[:, :])

        # pos chunk tiles: first 3 dedicated, rest rotate through the out
        # pool so each pos DMA waits (WAR) for an earlier out-store to drain
        # -- keeps the pos stream off the HBM wire during the x prelude
        for i in range(3):
            pt = singles.tile([C, xc], bf16, tag=f"pin{i}", name=f"pin{i}")
            pst.append(pt)

        def pos_dma(i):
            nc.gpsimd.dma_start(out=pst[i], in_=pos_d[:, i * xc:(i + 1) * xc])

        # ---- remaining pool levels (sums) + host-folded pos pool sums
        p8 = singles.tile([C, 8, 8], f32)
        nc.vector.reduce_sum(
            p8, s1.rearrange("c (hg hi) wg -> c hg wg hi", hi=16), axis=AX.X)
        p4 = singles.tile([C, 4, 4], f32)
        t44 = singles.tile([C, 8, 4], f32)
        nc.vector.reduce_sum(t44, p8.rearrange("c h (wg wi) -> c h wg wi", wi=2), axis=AX.X)
        nc.vector.reduce_sum(p4, t44.rearrange("c (hg hi) w -> c hg w hi", hi=2), axis=AX.X)
        p2 = singles.tile([C, 2, 2], f32)
        t22 = singles.tile([C, 4, 2], f32)
        nc.vector.reduce_sum(t22, p4.rearrange("c h (wg wi) -> c h wg wi", wi=2), axis=AX.X)
        nc.vector.reduce_sum(p2, t22.rearrange("c (hg hi) w -> c hg w hi", hi=2), axis=AX.X)
        p1 = singles.tile([C, 1, 1], f32)
        t11 = singles.tile([C, 2, 1], f32)
        nc.vector.reduce_sum(t11, p2.rearrange("c h (wg wi) -> c h wg wi", wi=2), axis=AX.X)
        nc.vector.reduce_sum(p1, t11.rearrange("c (hg hi) w -> c hg w hi", hi=2), axis=AX.X)
        offs = {8: 0, 4: 64, 2: 80, 1: 84}
        for s, ps in ((8, p8), (4, p4), (2, p2), (1, p1)):
            o = offs[s]
            psl = pp_sb[:, o:o + s * s].rearrange("c (h w) -> c h w", h=s)
            nc.vector.tensor_add(ps, ps, psl)

        # ---- fused depthwise+pointwise: z1 = sum_t PWdiag(tap_t) @ pad_shift_t
        # (per-level 1/blk scale dropped -- LN normalizes it out)
        z1_ps = ps_ro.tile([C, KV], f32, tag="ro")
        for lvl, (s, ps) in enumerate(((8, p8), (4, p4), (2, p2), (1, p1))):
            pad = singles.tile([C, (s + 2) * (s + 2)], bf16, tag=f"pad{s}")
            nc.vector.memset(pad, 0.0)
            pad3 = pad.rearrange("c (h w) -> c h w", h=s + 2)
            nc.vector.tensor_copy(pad3[:, 1:s + 1, 1:s + 1], ps)
            o = offs[s]
            dst = z1_ps[:, o:o + s * s].rearrange("c (h w) -> c h w", h=s)
            for di in range(3):
                for dj in range(3):
                    t = 3 * di + dj
                    nc.tensor.matmul(dst, lhsT=pwdw_sb[:, t * C:(t + 1) * C],
                                     rhs=pad3[:, di:di + s, dj:dj + s],
                                     start=(t == 0), stop=(t == 8))
        z1_sb = singles.tile([C, KV], f32)
        nc.scalar.copy(z1_sb, z1_ps)

        # ---- LN over c: transpose -> stats -> zn -> transpose back -> gelu
        zt_ps = ps_ro.tile([KV, C], f32, tag="ro")
        nc.tensor.transpose(zt_ps, z1_sb, idn_sb)
        zt_sb = singles.tile([KV, C], f32)
        nc.scalar.copy(zt_sb, zt_ps)
        nmu = singles.tile([KV, 1], f32)
        nc.vector.reduce_sum(nmu, zt_sb, axis=AX.X, negate=True)
        nc.vector.tensor_scalar_mul(nmu, nmu, 1.0 / C)
        zc = singles.tile([KV, C], f32)
        nc.vector.tensor_scalar_add(zc, zt_sb, nmu)
        sq = singles.tile([KV, C], f32)
        nc.vector.tensor_mul(sq, zc, zc)
        var = singles.tile([KV, 1], f32)
        nc.vector.reduce_sum(var, sq, axis=AX.X)
        std = singles.tile([KV, 1], f32)
        eps_sb = singles.tile([KV, 1], f32)
        nc.vector.memset(eps_sb, 1e-5)
        nc.scalar.activation(std, var, AF.Sqrt, bias=eps_sb, scale=1.0 / C)
        rstd = singles.tile([KV, 1], f32)
        nc.vector.reciprocal(rstd, std)
        zn = singles.tile([KV, C], f32)
        nc.vector.tensor_scalar_mul(zn, zc, rstd)
        znt_ps = ps_ro.tile([C, KV], f32, tag="ro")
        nc.tensor.transpose(znt_ps, zn, idn_sb[:KV, :KV])
        z2 = singles.tile([C, KV], bf16)
        nc.scalar.activation(z2, znt_ps, AF.Gelu, bias=lnb_sb, scale=lnw_sb)

        # ---- vkv (85, 128) bf16 + b2 blockdiag first (H MMs depend on it),
        # then A (c, 768 zero-padded)
        vt_ps = ps_ro.tile([C, KV], f32, tag="ro")
        nc.tensor.matmul(vt_ps, lhsT=wvt_sb, rhs=z2, start=True, stop=True)
        vt_sb = singles.tile([C, KV], f32)
        nc.scalar.copy(vt_sb, vt_ps)
        vkv_ps = ps_ro.tile([KV, C], f32, tag="ro")
        nc.tensor.transpose(vkv_ps, vt_sb, idn_sb)
        vkv_sb = singles.tile([KV, C], f8)
        nc.scalar.copy(vkv_sb, vkv_ps)

        b2_sb = singles.tile([C, NCH * C], f8)
        nc.vector.memset(b2_sb, 0.0)
        nq = 0
        for m in range(M):
            g0, g1 = KV * m, KV * (m + 1)
            for cchunk in range(NCH):
                c0, c1 = CH_B[cchunk], CH_B[cchunk + 1]
                lo, hi = max(g0, c0), min(g1, c1)
                if lo >= hi:
                    continue
                nc.gpsimd.dma_start(
                    out=b2_sb[lo - c0:hi - c0,
                              cchunk * C + HD * m: cchunk * C + HD * m + HD],
                    in_=vkv_sb[lo - g0:hi - g0, HD * m:HD * m + HD])
                nq += 1

        a_sb = singles.tile([C, NCH * C], bf16)
        nc.vector.memset(a_sb[:, KVH:], 0.0)
        for half in range(2):
            a_ps = ps_ro.tile([C, 4 * KV], f32, tag="ro")
            for mi in range(4):
                m = half * 4 + mi
                nc.tensor.matmul(a_ps[:, mi * KV:(mi + 1) * KV],
                                 lhsT=ct_sb[:, m * C:(m + 1) * C], rhs=z2,
                                 start=True, stop=True)
            nc.scalar.copy(a_sb[:, half * 4 * KV:(half + 1) * 4 * KV], a_ps)

        # ---- main attention loop, flat pipeline over 96 chunk-pairs:
        # scores/exp of pair P are emitted BEFORE H/r of pair P-1, so the PE
        # FIFO always has ready score work while ACT runs exp. Wproj/store
        # tail runs one group behind; xp adds stream into iterations 0..15.
        xp_sb = xpool.tile([C, HW], bf16)
        for i in range(3):
            pos_dma(i)
        GH = 1024
        NP = 3 * NG
        gstate = {}
        pend = [None]

        def emit_tail2():
            hn_, tt = pend[0]
            o_ps = ps_ro.tile([C, G], f32, tag="ro")
            nc.tensor.matmul(o_ps, lhsT=wpt_sb, rhs=hn_, start=True, stop=True)
            o_sb = outp.tile([C, G], f32)
            nc.vector.tensor_scalar_add(o_sb, o_ps, bpj_sb)
            nc.gpsimd.dma_start(out=out_d[:, tt:tt + G], in_=o_sb)
            pend[0] = None

        def emit_scores_exp(P):
            g, pp = divmod(P, 3)
            if pp == 0:
                if g + 3 < 16:
                    pst.append(outp.tile([C, xc], bf16, tag="o",
                                         name=f"pin{g + 3}"))
                    pos_dma(g + 3)
                # one 512-token add per group keeps DVE load flat at
                # ~0.7us/group instead of 1.2us spikes in the first half
                nc.vector.tensor_add(xp_sb[:, g * G:(g + 1) * G],
                                     xst[g // 2][:, (g % 2) * G:(g % 2 + 1) * G],
                                     pst[g // 2][:, (g % 2) * G:(g % 2 + 1) * G])
                gstate[g] = (ps_h.tile([C, G], f32, tag="h", name=f"h{g}"),
                             ps_r.tile([C, G], f32, tag="r", name=f"r{g}"),
                             exp_pool.tile([C, NCH * G], f8, tag="ex",
                                           name=f"ex{g}"))
            h_ps, r_ps, ex_sb = gstate[g]
            xg = xp_sb[:, g * G:(g + 1) * G]
            s_ps = ps_sc.tile([C, 2 * G], f32, tag="s")
            for j in range(2):
                cc = 2 * pp + j
                nc.tensor.matmul(s_ps[:, j * G:(j + 1) * G],
                                 lhsT=a_sb[:, cc * C:(cc + 1) * C], rhs=xg,
                                 start=True, stop=True)
            if pp == 1 and pend[0] is not None:
                emit_tail2()
            nc.scalar.activation(ex_sb[:, 2 * pp * G:(2 * pp + 2) * G], s_ps,
                                 AF.Exp)

        def emit_hr(P):
            g, pp = divmod(P, 3)
            h_ps, r_ps, ex_sb = gstate[g]
            erhs = ex_sb[:, 2 * pp * G:(2 * pp + 2) * G].rearrange(
                "k (two t) -> k two t", two=2)
            bv = b2_sb[:, 2 * pp * C:(2 * pp + 2) * C].rearrange(
                "k (two m) -> k two m", two=2)
            ov = ones_sb[:, 2 * pp * C:(2 * pp + 2) * C].rearrange(
                "k (two m) -> k two m", two=2)
            nc.tensor.matmul(h_ps, lhsT=bv, rhs=erhs, perf_mode=DR,
                             start=(pp == 0), stop=(pp == 2))
            nc.tensor.matmul(r_ps, lhsT=ov, rhs=erhs, perf_mode=DR,
                             start=(pp == 0), stop=(pp == 2))
            if pp == 2:
                rec = rr_pool.tile([C, G], f32, tag="rec")
                nc.vector.reciprocal_approx_fast(rec, r_ps)
                hn = rr_pool.tile([C, G], bf16, tag="hn")
                nc.vector.tensor_mul(hn, h_ps, rec)
                pend[0] = (hn, g * G)
                del gstate[g]

        for P in range(NP + 1):
            if P < NP:
                emit_scores_exp(P)
            if P >= 1:
                emit_hr(P - 1)
        emit_tail2()

    nc.finalize()
    return nc


def _consts(Wq, Wkv, Wproj, bproj, dw_w, pw_w, ln_w, ln_b):
    import ml_dtypes

    bf16 = ml_dtypes.bfloat16
    scale = HD ** -0.5
    Wk, Wv = Wkv[:128], Wkv[128:]
    ct = np.zeros((M * C, C), np.float32)
    for m in range(M):
        ct[m * C:(m + 1) * C] = scale * Wk[16 * m:16 * m + 16].T @ Wq[16 * m:16 * m + 16]
    # fused pointwise*diag(dw tap t), transposed for lhsT
    pw = pw_w[:, :, 0, 0]                      # (C out, C in)
    taps = dw_w[:, 0].reshape(C, 9)            # (C, 9)
    pwdw = np.zeros((9 * C, C), np.float32)
    for t in range(9):
        pwdw[t * C:(t + 1) * C] = pw.T * taps[:, t:t + 1]
    # pos pool sums per level, level-major like dwcat
    pos = _pos_full().reshape(C, Hh, Ww)
    pospool = np.zeros((C, KV), np.float32)
    offs = {8: 0, 4: 64, 2: 80, 1: 84}
    for s in (8, 4, 2, 1):
        blk = Hh // s
        psum = pos.reshape(C, s, blk, s, blk).sum((2, 4))
        pospool[:, offs[s]:offs[s] + s * s] = psum.reshape(C, s * s)
    # expanded ones: chunk cc row r hits all 16 channels of its head
    onesb = np.zeros((C, NCH * C), np.float32)
    for cc in range(NCH):
        c0, c1 = CH_B[cc], CH_B[cc + 1]
        for r in range(c1 - c0):
            m = (c0 + r) // KV
            onesb[r, cc * C + HD * m: cc * C + HD * (m + 1)] = 1.0
    return {
        "ct": ct.astype(bf16),
        "wvt": np.ascontiguousarray(Wv.T).astype(bf16),
        "pwdw": pwdw.astype(bf16),
        "wpt": np.ascontiguousarray(Wproj.T).astype(bf16),
        "pospool": pospool,
        "onesb": onesb.astype(ml_dtypes.float8_e4m3),
        "lnw": ln_w.reshape(C, 1).astype(np.float32),
        "lnb": ln_b.reshape(C, 1).astype(np.float32),
        "bpj": bproj.reshape(C, 1).astype(np.float32),
        "idn": np.eye(C, dtype=np.float32),
        "pos": _pos_full().astype(bf16),
    }


def kernel(x, Wq, Wkv, Wproj, bproj, dw_w, pw_w, ln_w, ln_b):
    from concourse.bass_utils import run_bass_kernel_spmd

    if "nc" not in _CACHE:
        _CACHE["nc"] = _build()
    nc = _CACHE["nc"]

    cst = _consts(np.asarray(Wq, np.float32), np.asarray(Wkv, np.float32),
                  np.asarray(Wproj, np.float32), np.asarray(bproj, np.float32),
                  np.asarray(dw_w, np.float32), np.asarray(pw_w, np.float32),
                  np.asarray(ln_w, np.float32), np.asarray(ln_b, np.float32))
    x = np.asarray(x, np.float32)
    in_maps = []
    for b in range(B):
        im = {"x": np.ascontiguousarray(x[b].reshape(C, HW))}
        im.update(cst)
        in_maps.append(im)

    trace = bool(int(os.environ.get("KPROF", "0")))
    res = run_bass_kernel_spmd(nc, in_maps, core_ids=list(range(B)), trace=trace)
    if trace and res.exec_time_ns is not None:
        print(f"HW exec time: {res.exec_time_ns} ns")
    out = np.stack([res.results[b]["out"].reshape(C, Hh, Ww) for b in range(B)])
    return out
